# revision 10
# baseline (speedup 1.0000x reference)
"""Cantor global attention kernel for Trainium2 (8 NeuronCores, SPMD).

Strategy: data-parallel over batch B=64 -> 8 cores x 8 rows each.
All device tensors are 16-bit: Q/K (and the pre-exp score t) in fp16
for exponent accuracy, everything after the exp in bf16 for range
safety (scores reach ~|20| so e^t needs bf16's exponent range).  The
host uploads inputs already transposed into the SBUF layout
[proj][128 part][e*256 col] so every DMA is a few large contiguous
descriptors, and converts the bf16 output back to f32.

Per core, partition = b*16 + p//256; each expert owns 256 columns;
the W=3 neighbor gather becomes column offsets baked from the runtime
routes (slot-permuted so slot0 = self).

Engine placement (per core):
  - projection averaging (Q,K,V): DMA-accumulate (CCE add) - free
  - t_w = Qs*Ks_route:  DVE tensor_mul fp16 (2x packed), run-batched
  - gate: t *= sigmoid(beta) per non-self (e,w): DVE tensor_scalar
    (4x mode), immediate baked from betas
  - e_w = exp(esc*t):   ScalarE activation, uniform scale immediate
    esc = 0.25/(sqrt(128)*|temp|)  ->  fully batched big instructions
  - prod_w = e_w*Vs:    DVE tensor_mul bf16, run-batched
  - den|num = sum_w:    2 DVE adds per group over a [k=2,w=3] strided
    view covering both reductions
  - r = 0.5/den:        ScalarE ln (fp32 out) then exp(-x+ln 0.5)
  - out = num*r:        DVE mul, stored bf16
"""

import math

import numpy as np

import concourse.bass as bass
import concourse.mybir as mybir
from concourse import bacc, tile
from concourse.bass_utils import run_bass_kernel_spmd

E, NPROJ, B, P = 16, 2, 64, 4096
W = 3
EXPERT_DIM = 128
NCORES = 8
BS = B // NCORES          # 8 batch rows per core
COLS = 256                # free-dim columns per expert slab
PH = P // COLS            # 16 partition sub-blocks per batch row
PART = BS * PH            # 128 SBUF partitions
EC = E * COLS             # 4096 cols per w-block
GROUP = 4                 # experts per compute group
NG = E // GROUP           # 4 groups
GC = GROUP * COLS         # 1024 cols per group
ACT_SET_LN_EXP = 6        # act_info.json natural_log_exp_and_others

F16 = mybir.dt.float16
BF16 = mybir.dt.bfloat16
F32 = mybir.dt.float32
EXPF = mybir.ActivationFunctionType.Exp
LNF = mybir.ActivationFunctionType.Ln
MULT = mybir.AluOpType.mult
ADD = mybir.AluOpType.add


def _runs(pairs):
    """Split [(e, j), ...] into maximal runs of consecutive e and j."""
    runs = []
    for e, j in pairs:
        if runs and runs[-1][0] + runs[-1][2] == e and runs[-1][1] + runs[-1][2] == j:
            runs[-1][2] += 1
        else:
            runs.append([e, j, 1])
    return runs


def _build_nc(routes_s: np.ndarray, gates_s: np.ndarray, esc: float):
    nc = bacc.Bacc("TRN2", target_bir_lowering=False, debug=False,
                   num_devices=NCORES)

    q_d = nc.dram_tensor("q", [NPROJ, PART, EC], F16, kind="ExternalInput")
    k_d = nc.dram_tensor("k", [NPROJ, PART, EC], F16, kind="ExternalInput")
    v_d = nc.dram_tensor("v", [NPROJ, PART, EC], F16, kind="ExternalInput")
    o_d = nc.dram_tensor("out", [PART, EC], BF16, kind="ExternalOutput")

    # per-group, per-slot runs split at group boundaries
    runs_g = [[] for _ in range(NG)]
    for g in range(NG):
        for w in range(W):
            pairs = [(e, int(routes_s[e, w]))
                     for e in range(g * GROUP, (g + 1) * GROUP)]
            for e0, j0, L in _runs(pairs):
                runs_g[g].append((w, e0, j0, L))

    with tile.TileContext(nc) as tc:
        with (
            tc.tile_pool(name="io", bufs=1) as io_p,
            tc.tile_pool(name="mid", bufs=1) as mid_p,
        ):
            qs = io_p.tile([PART, EC], F16, name="qs", tag="qs")
            ks = io_p.tile([PART, EC], F16, name="ks", tag="ks")
            vs = io_p.tile([PART, EC], F16, name="vs", tag="vs")
            kraw = io_p.tile([PART, EC], F16, name="kraw", tag="kraw")
            qraw = io_p.tile([PART, EC], F16, name="qraw", tag="qraw")
            tp = mid_p.tile([PART, W * EC], F16, name="tp", tag="tp")
            epr = mid_p.tile([PART, 2 * W * EC], BF16, name="epr", tag="epr")
            dn = mid_p.tile([PART, 2 * EC], BF16, name="dn", tag="dn")
            lnt = mid_p.tile([PART, EC], F32, name="lnt", tag="lnt")
            rr = mid_p.tile([PART, EC], BF16, name="rr", tag="rr")
            og = mid_p.tile([PART, EC], BF16, name="og", tag="og")

            qv, kv, vv = q_d.ap(), k_d.ap(), v_d.ap()
            ov = o_d.ap()

            H = EC // 2

            def load_cce(dst, src, half, ring):
                """proj0 plain + proj1 DMA-accumulate (needs warm Q7)."""
                c0, c1 = half * H, (half + 1) * H
                ring.dma_start(dst[:, c0:c1], src[0][:, c0:c1])
                nc.gpsimd.dma_start(dst[:, c0:c1], src[1][:, c0:c1],
                                    accum_op=ADD)

            def load_raw(raw, dst, src, half, ring):
                """Both projections to a raw tile + DVE add - no Q7 on the
                critical path for the first compute group."""
                c0, c1 = half * H, (half + 1) * H
                ring.dma_start(raw[:, 0:H], src[0][:, c0:c1])
                ring.dma_start(raw[:, H:2 * H], src[1][:, c0:c1])
                nc.vector.tensor_add(dst[:, c0:c1], raw[:, 0:H],
                                     raw[:, H:2 * H])

            # strided views
            tpv = tp[:].rearrange("p (w c) -> p w c", w=W)
            epv = epr[:].rearrange("p (k w c) -> p k w c", k=2, w=W)
            dnv = dn[:].rearrange("p (k c) -> p k c", k=2)

            def compute(g):
                c0, c1 = g * GC, (g + 1) * GC
                # t = Qs * Ks[route]
                for w, e0, j0, L in runs_g[g]:
                    nc.vector.tensor_mul(
                        tp[:, w * EC + e0 * COLS: w * EC + (e0 + L) * COLS],
                        qs[:, e0 * COLS:(e0 + L) * COLS],
                        ks[:, j0 * COLS:(j0 + L) * COLS])
                # gate the non-self slots (slot0 is self, gate 1)
                for w in range(1, W):
                    for e in range(g * GROUP, (g + 1) * GROUP):
                        sl = slice(w * EC + e * COLS, w * EC + (e + 1) * COLS)
                        nc.vector.tensor_scalar_mul(
                            tp[:, sl], tp[:, sl], float(gates_s[e, w]))
                # e = exp(esc * t), one instruction per group over all 3 slots
                nc.scalar.activation(epv[:, 0, :, c0:c1], tpv[:, :, c0:c1],
                                     EXPF, bias=0.0, scale=esc)
                # prod = e * Vs[route]
                for w, e0, j0, L in runs_g[g]:
                    nc.vector.tensor_mul(
                        epr[:, (W + w) * EC + e0 * COLS:
                            (W + w) * EC + (e0 + L) * COLS],
                        epr[:, w * EC + e0 * COLS: w * EC + (e0 + L) * COLS],
                        vs[:, j0 * COLS:(j0 + L) * COLS])
                # den | num sums over w in two adds
                nc.vector.tensor_add(dnv[:, :, c0:c1], epv[:, :, 0, c0:c1],
                                     epv[:, :, 1, c0:c1])
                nc.vector.tensor_add(dnv[:, :, c0:c1], dnv[:, :, c0:c1],
                                     epv[:, :, 2, c0:c1])
                # r = 0.5/den = exp(-ln(2*den)); the 0.5 rides the ln scale
                nc.scalar.activation(lnt[:, c0:c1], dn[:, c0:c1], LNF,
                                     bias=0.0, scale=2.0)
                nc.scalar.activation(rr[:, c0:c1], lnt[:, c0:c1], EXPF,
                                     bias=0.0, scale=-1.0)
                # out = num * r
                nc.vector.tensor_mul(og[:, c0:c1], dn[:, EC + c0:EC + c1],
                                     rr[:, c0:c1])

            def store(half):
                c0, c1 = half * H, (half + 1) * H
                nc.sync.dma_start(ov[:, c0:c1], og[:, c0:c1])

            # Wave 1 (critical path, no Q7): raw loads + DVE averaging for
            # the k/q halves that unblock group 3; V rides CCE since prod
            # starts an exp later.  Later halves use CCE accumulate - by
            # then the Q7 warmup (~6us, const-AP memsets) has happened.
            load_raw(kraw, ks, kv, 1, nc.sync)
            load_raw(qraw, qs, qv, 1, nc.scalar)
            # pin the exp+ln ACT table once the scalar ring's DMAs are out
            nc.scalar.add_instruction(mybir.InstLoadActFuncSet(
                name=nc.get_next_instruction_name(),
                act_func_set_id=ACT_SET_LN_EXP, ins=[], outs=[]))
            load_cce(vs, vv, 1, nc.sync)
            load_cce(ks, kv, 0, nc.sync)
            load_cce(qs, qv, 0, nc.scalar)
            load_cce(vs, vv, 0, nc.sync)
            compute(3)
            compute(2)
            store(1)
            compute(0)
            compute(1)
            store(0)

    nc.compile()
    return nc


_cache: dict = {}


def _get_nc(routes_s: np.ndarray, gates_s: np.ndarray, esc: float):
    key = (routes_s.tobytes(), gates_s.tobytes(), float(esc))
    if key not in _cache:
        _cache[key] = _build_nc(routes_s, gates_s, esc)
    return _cache[key]


def _slot_sort(routes: np.ndarray, betas: np.ndarray):
    """Slot-permute so slot0 = self (gate 1); others sorted by offset."""
    gate = np.where(routes != np.arange(E, dtype=np.int32)[:, None],
                    1.0 / (1.0 + np.exp(-betas.astype(np.float64))),
                    1.0)
    routes_s = np.zeros((E, W), np.int32)
    gates_s = np.ones((E, W), np.float64)
    for e in range(E):
        slots = list(range(W))
        self_w = [w for w in slots if routes[e, w] == e]
        assert self_w, f"expert {e} missing self route"
        rest = [w for w in slots if w != self_w[0]]
        rest.sort(key=lambda w: int(routes[e, w]) - e)
        order = [self_w[0]] + rest
        routes_s[e] = routes[e, order]
        gates_s[e] = gate[e, order]
    return routes_s, gates_s.astype(np.float32)


def kernel(Q_proj, K_proj, V_proj, betas, temperature, routes, num_patches):
    Q = np.asarray(Q_proj, dtype=np.float32)
    K = np.asarray(K_proj, dtype=np.float32)
    V = np.asarray(V_proj, dtype=np.float32)
    betas = np.asarray(betas, dtype=np.float32)
    temp = np.asarray(temperature, dtype=np.float32)
    routes = np.asarray(routes, dtype=np.int32)
    assert int(num_patches) == E * P

    # Qs = Q0+Q1 (2x the mean); the 0.25 from both means is folded into
    # the exp scale esc together with sqrt(d)*|temperature|.
    esc = float(0.25 / (np.sqrt(np.float32(EXPERT_DIM)) * np.abs(temp[0])))
    routes_s, gates_s = _slot_sort(routes, betas)
    nc = _get_nc(routes_s, gates_s, esc)

    def prep(X):
        # [E, NPROJ, BS, P] -> [NPROJ, (b ph), (e c)] fp16
        return np.ascontiguousarray(
            X.reshape(E, NPROJ, BS, PH, COLS).transpose(1, 2, 3, 0, 4)
            .reshape(NPROJ, PART, EC).astype(np.float16))

    in_maps = []
    for c in range(NCORES):
        sl = slice(c * BS, (c + 1) * BS)
        in_maps.append({
            "q": prep(Q[:, :, sl, :]),
            "k": prep(K[:, :, sl, :]),
            "v": prep(V[:, :, sl, :]),
        })

    res = run_bass_kernel_spmd(nc, in_maps, list(range(NCORES)))
    out = np.empty((B, E * P), np.float32)
    for c in range(NCORES):
        o = np.asarray(res.results[c]["out"]).astype(np.float32)
        out[c * BS:(c + 1) * BS] = (
            o.reshape(BS, PH, E, COLS).transpose(0, 2, 1, 3)
            .reshape(BS, E * P))
    return out


# revision 14
# speedup vs baseline: 1.0483x; 1.0483x over previous
"""Cantor global attention kernel for Trainium2 (8 NeuronCores, SPMD).

Strategy: data-parallel over batch B=64 -> 8 cores x 8 rows each.
All device tensors are 16-bit: Q/K (and the pre-exp score t) in fp16
for exponent accuracy, everything after the exp in bf16 for range
safety (scores reach ~|20| so e^t needs bf16's exponent range).  The
host uploads inputs already transposed into the SBUF layout
[proj][128 part][e*256 col] so every DMA is a few large contiguous
descriptors, and converts the bf16 output back to f32.

Per core, partition = b*16 + p//256; each expert owns 256 columns;
the W=3 neighbor gather becomes column offsets baked from the runtime
routes (slot-permuted so slot0 = self).

Engine placement (per core):
  - projection averaging (Q,K,V): DMA-accumulate (CCE add) - free
  - t_w = Qs*Ks_route:  DVE tensor_mul fp16 (2x packed), run-batched
  - gate: t *= sigmoid(beta) per non-self (e,w): DVE tensor_scalar
    (4x mode), immediate baked from betas
  - e_w = exp(esc*t):   ScalarE activation, uniform scale immediate
    esc = 0.25/(sqrt(128)*|temp|)  ->  fully batched big instructions
  - prod_w = e_w*Vs:    DVE tensor_mul bf16, run-batched
  - den|num = sum_w:    2 DVE adds per group over a [k=2,w=3] strided
    view covering both reductions
  - r = 0.5/den:        ScalarE ln (fp32 out) then exp(-x+ln 0.5)
  - out = num*r:        DVE mul, stored bf16
"""

import math

import numpy as np

import concourse.bass as bass
import concourse.mybir as mybir
from concourse import bacc, tile
from concourse.bass_utils import run_bass_kernel_spmd

E, NPROJ, B, P = 16, 2, 64, 4096
W = 3
EXPERT_DIM = 128
NCORES = 8
BS = B // NCORES          # 8 batch rows per core
COLS = 256                # free-dim columns per expert slab
PH = P // COLS            # 16 partition sub-blocks per batch row
PART = BS * PH            # 128 SBUF partitions
EC = E * COLS             # 4096 cols per w-block
GROUP = 4                 # experts per compute group
NG = E // GROUP           # 4 groups
GC = GROUP * COLS         # 1024 cols per group
ACT_SET_LN_EXP = 6        # act_info.json natural_log_exp_and_others

F16 = mybir.dt.float16
BF16 = mybir.dt.bfloat16
F32 = mybir.dt.float32
EXPF = mybir.ActivationFunctionType.Exp
LNF = mybir.ActivationFunctionType.Ln
MULT = mybir.AluOpType.mult
ADD = mybir.AluOpType.add


def _runs(pairs):
    """Split [(e, j), ...] into maximal runs of consecutive e and j."""
    runs = []
    for e, j in pairs:
        if runs and runs[-1][0] + runs[-1][2] == e and runs[-1][1] + runs[-1][2] == j:
            runs[-1][2] += 1
        else:
            runs.append([e, j, 1])
    return runs


def _build_nc(routes_s: np.ndarray, gates_s: np.ndarray, esc: float):
    nc = bacc.Bacc("TRN2", target_bir_lowering=False, debug=False,
                   num_devices=NCORES)

    q_d = nc.dram_tensor("q", [NPROJ, PART, EC], F16, kind="ExternalInput")
    k_d = nc.dram_tensor("k", [NPROJ, PART, EC], F16, kind="ExternalInput")
    v_d = nc.dram_tensor("v", [NPROJ, PART, EC], F16, kind="ExternalInput")
    o_d = nc.dram_tensor("out", [PART, EC], BF16, kind="ExternalOutput")

    # per-group, per-slot runs split at group boundaries
    runs_g = [[] for _ in range(NG)]
    for g in range(NG):
        for w in range(W):
            pairs = [(e, int(routes_s[e, w]))
                     for e in range(g * GROUP, (g + 1) * GROUP)]
            for e0, j0, L in _runs(pairs):
                runs_g[g].append((w, e0, j0, L))

    with tile.TileContext(nc) as tc:
        with (
            tc.tile_pool(name="io", bufs=1) as io_p,
            tc.tile_pool(name="mid", bufs=1) as mid_p,
        ):
            qs = io_p.tile([PART, EC], F16, name="qs", tag="qs")
            ks = io_p.tile([PART, EC], F16, name="ks", tag="ks")
            vs = io_p.tile([PART, EC], F16, name="vs", tag="vs")
            kraw = io_p.tile([PART, EC], F16, name="kraw", tag="kraw")
            qraw = io_p.tile([PART, EC], F16, name="qraw", tag="qraw")
            tp = mid_p.tile([PART, W * EC], F16, name="tp", tag="tp")
            epr = mid_p.tile([PART, 2 * W * EC], BF16, name="epr", tag="epr")
            dn = mid_p.tile([PART, 2 * EC], BF16, name="dn", tag="dn")
            lnt = mid_p.tile([PART, EC], F32, name="lnt", tag="lnt")
            rr = mid_p.tile([PART, EC], BF16, name="rr", tag="rr")
            og = mid_p.tile([PART, EC], BF16, name="og", tag="og")

            qv, kv, vv = q_d.ap(), k_d.ap(), v_d.ap()
            ov = o_d.ap()

            H = EC // 2
            crit_loads = []

            def load_cce(dst, src, half, ring):
                """proj0 plain + proj1 DMA-accumulate (needs warm Q7).
                Gated behind the critical first-wave loads so they don't
                pollute the DMA queues / completion semaphores."""
                c0, c1 = half * H, (half + 1) * H
                i0 = ring.dma_start(dst[:, c0:c1], src[0][:, c0:c1])
                for gi in crit_loads:
                    tile.add_dep_helper(i0.ins, gi.ins, sync=True,
                                        reason="load wave gating")
                nc.gpsimd.dma_start(dst[:, c0:c1], src[1][:, c0:c1],
                                    accum_op=ADD)

            def load_crit(raw, dst, src, ring):
                """Single DMA for both projections of the h1 half + DVE
                add - fastest possible path to the first compute group
                (no Q7 warmup, one completion semaphore)."""
                rv = raw[:].rearrange("p (n c) -> p n c", n=NPROJ)
                sv = src.rearrange("n p (h c) -> p h n c", h=2)[:, 1]
                i0 = ring.dma_start(rv, sv)
                crit_loads.append(i0)
                nc.vector.tensor_add(dst[:, H:EC], raw[:, 0:H],
                                     raw[:, H:2 * H])

            # strided views
            tpv = tp[:].rearrange("p (w c) -> p w c", w=W)
            epv = epr[:].rearrange("p (k w c) -> p k w c", k=2, w=W)
            dnv = dn[:].rearrange("p (k c) -> p k c", k=2)

            def compute(g, fin_chunks=1):
                # t = Qs * Ks[route]
                for w, e0, j0, L in runs_g[g]:
                    nc.vector.tensor_mul(
                        tp[:, w * EC + e0 * COLS: w * EC + (e0 + L) * COLS],
                        qs[:, e0 * COLS:(e0 + L) * COLS],
                        ks[:, j0 * COLS:(j0 + L) * COLS])
                # gate the non-self slots (slot0 is self, gate 1)
                for w in range(1, W):
                    for e in range(g * GROUP, (g + 1) * GROUP):
                        sl = slice(w * EC + e * COLS, w * EC + (e + 1) * COLS)
                        nc.vector.tensor_scalar_mul(
                            tp[:, sl], tp[:, sl], float(gates_s[e, w]))
                # e = exp(esc * t), one instruction per group over all 3 slots
                gc0, gc1 = g * GC, (g + 1) * GC
                nc.scalar.activation(epv[:, 0, :, gc0:gc1], tpv[:, :, gc0:gc1],
                                     EXPF, bias=0.0, scale=esc)
                # prod = e * Vs[route]
                for w, e0, j0, L in runs_g[g]:
                    nc.vector.tensor_mul(
                        epr[:, (W + w) * EC + e0 * COLS:
                            (W + w) * EC + (e0 + L) * COLS],
                        epr[:, w * EC + e0 * COLS: w * EC + (e0 + L) * COLS],
                        vs[:, j0 * COLS:(j0 + L) * COLS])
                fc = GC // fin_chunks
                for f in range(fin_chunks):
                    c0, c1 = gc0 + f * fc, gc0 + (f + 1) * fc
                    # den | num sums over w in two adds
                    nc.vector.tensor_add(dnv[:, :, c0:c1],
                                         epv[:, :, 0, c0:c1],
                                         epv[:, :, 1, c0:c1])
                    nc.vector.tensor_add(dnv[:, :, c0:c1], dnv[:, :, c0:c1],
                                         epv[:, :, 2, c0:c1])
                    # r = 0.5/den = exp(-ln(2*den)); 0.5 rides the ln scale
                    nc.scalar.activation(lnt[:, c0:c1], dn[:, c0:c1], LNF,
                                         bias=0.0, scale=2.0)
                    nc.scalar.activation(rr[:, c0:c1], lnt[:, c0:c1], EXPF,
                                         bias=0.0, scale=-1.0)
                    # out = num * r
                    nc.vector.tensor_mul(og[:, c0:c1],
                                         dn[:, EC + c0:EC + c1],
                                         rr[:, c0:c1])

            def store(c0, c1, ring=None):
                (ring or nc.sync).dma_start(ov[:, c0:c1], og[:, c0:c1])

            # Wave 1 (critical path, no Q7): one-DMA raw loads + DVE
            # averaging for the k/q halves that unblock group 3; V rides
            # CCE since prod starts an exp later.  Later halves use CCE
            # accumulate - by then the Q7 warmup (~6us) has happened -
            # and are gated behind wave 1.
            load_crit(kraw, ks, kv, nc.sync)
            load_crit(qraw, qs, qv, nc.scalar)
            # pin the exp+ln ACT table once the scalar ring's DMA is out
            nc.scalar.add_instruction(mybir.InstLoadActFuncSet(
                name=nc.get_next_instruction_name(),
                act_func_set_id=ACT_SET_LN_EXP, ins=[], outs=[]))
            load_cce(vs, vv, 1, nc.sync)
            load_cce(ks, kv, 0, nc.sync)
            load_cce(qs, qv, 0, nc.scalar)
            load_cce(vs, vv, 0, nc.sync)
            compute(3)
            compute(2)
            store(2 * GC, 4 * GC)
            compute(0)
            store(0, GC, nc.scalar)
            compute(1, fin_chunks=2)
            store(GC, GC + GC // 2)
            store(GC + GC // 2, 2 * GC, nc.scalar)

    nc.compile()
    return nc


_cache: dict = {}


def _get_nc(routes_s: np.ndarray, gates_s: np.ndarray, esc: float):
    key = (routes_s.tobytes(), gates_s.tobytes(), float(esc))
    if key not in _cache:
        _cache[key] = _build_nc(routes_s, gates_s, esc)
    return _cache[key]


def _slot_sort(routes: np.ndarray, betas: np.ndarray):
    """Slot-permute so slot0 = self (gate 1); others sorted by offset."""
    gate = np.where(routes != np.arange(E, dtype=np.int32)[:, None],
                    1.0 / (1.0 + np.exp(-betas.astype(np.float64))),
                    1.0)
    routes_s = np.zeros((E, W), np.int32)
    gates_s = np.ones((E, W), np.float64)
    for e in range(E):
        slots = list(range(W))
        self_w = [w for w in slots if routes[e, w] == e]
        assert self_w, f"expert {e} missing self route"
        rest = [w for w in slots if w != self_w[0]]
        rest.sort(key=lambda w: int(routes[e, w]) - e)
        order = [self_w[0]] + rest
        routes_s[e] = routes[e, order]
        gates_s[e] = gate[e, order]
    return routes_s, gates_s.astype(np.float32)


def kernel(Q_proj, K_proj, V_proj, betas, temperature, routes, num_patches):
    Q = np.asarray(Q_proj, dtype=np.float32)
    K = np.asarray(K_proj, dtype=np.float32)
    V = np.asarray(V_proj, dtype=np.float32)
    betas = np.asarray(betas, dtype=np.float32)
    temp = np.asarray(temperature, dtype=np.float32)
    routes = np.asarray(routes, dtype=np.int32)
    assert int(num_patches) == E * P

    # Qs = Q0+Q1 (2x the mean); the 0.25 from both means is folded into
    # the exp scale esc together with sqrt(d)*|temperature|.
    esc = float(0.25 / (np.sqrt(np.float32(EXPERT_DIM)) * np.abs(temp[0])))
    routes_s, gates_s = _slot_sort(routes, betas)
    nc = _get_nc(routes_s, gates_s, esc)

    def prep(X):
        # [E, NPROJ, BS, P] -> [NPROJ, (b ph), (e c)] fp16
        return np.ascontiguousarray(
            X.reshape(E, NPROJ, BS, PH, COLS).transpose(1, 2, 3, 0, 4)
            .reshape(NPROJ, PART, EC).astype(np.float16))

    in_maps = []
    for c in range(NCORES):
        sl = slice(c * BS, (c + 1) * BS)
        in_maps.append({
            "q": prep(Q[:, :, sl, :]),
            "k": prep(K[:, :, sl, :]),
            "v": prep(V[:, :, sl, :]),
        })

    res = run_bass_kernel_spmd(nc, in_maps, list(range(NCORES)))
    out = np.empty((B, E * P), np.float32)
    for c in range(NCORES):
        o = np.asarray(res.results[c]["out"]).astype(np.float32)
        out[c * BS:(c + 1) * BS] = (
            o.reshape(BS, PH, E, COLS).transpose(0, 2, 1, 3)
            .reshape(BS, E * P))
    return out


# revision 19
# speedup vs baseline: 1.0672x; 1.0180x over previous
"""Cantor global attention kernel for Trainium2 (8 NeuronCores, SPMD).

Strategy: data-parallel over batch B=64 -> 8 cores x 8 rows each.
All device tensors are 16-bit: Q/K (and the pre-exp score t) in fp16
for exponent accuracy, everything after the exp in bf16 for range
safety (scores reach ~|20| so e^t needs bf16's exponent range).  The
host uploads inputs already transposed into the SBUF layout
[proj][128 part][e*256 col] so every DMA is a few large contiguous
descriptors, and converts the bf16 output back to f32.

Per core, partition = b*16 + p//256; each expert owns 256 columns;
the W=3 neighbor gather becomes column offsets baked from the runtime
routes (slot-permuted so slot0 = self).

Engine placement (per core):
  - projection averaging (Q,K,V): DMA-accumulate (CCE add) - free
  - t_w = Qs*Ks_route:  DVE tensor_mul fp16 (2x packed), run-batched
  - gate: t *= sigmoid(beta) per non-self (e,w): DVE tensor_scalar
    (4x mode), immediate baked from betas
  - e_w = exp(esc*t):   ScalarE activation, uniform scale immediate
    esc = 0.25/(sqrt(128)*|temp|)  ->  fully batched big instructions
  - prod_w = e_w*Vs:    DVE tensor_mul bf16, run-batched
  - den|num = sum_w:    2 DVE adds per group over a [k=2,w=3] strided
    view covering both reductions
  - r = 0.5/den:        ScalarE ln (fp32 out) then exp(-x+ln 0.5)
  - out = num*r:        DVE mul, stored bf16
"""

import math

import numpy as np

import concourse.bass as bass
import concourse.mybir as mybir
from concourse import bacc, tile
from concourse.bass_utils import run_bass_kernel_spmd

E, NPROJ, B, P = 16, 2, 64, 4096
W = 3
EXPERT_DIM = 128
NCORES = 8
BS = B // NCORES          # 8 batch rows per core
COLS = 256                # free-dim columns per expert slab
PH = P // COLS            # 16 partition sub-blocks per batch row
PART = BS * PH            # 128 SBUF partitions
EC = E * COLS             # 4096 cols per w-block
GROUP = 4                 # experts per compute group
NG = E // GROUP           # 4 groups
GC = GROUP * COLS         # 1024 cols per group
ACT_SET_LN_EXP = 6        # act_info.json natural_log_exp_and_others

F16 = mybir.dt.float16
BF16 = mybir.dt.bfloat16
F32 = mybir.dt.float32
EXPF = mybir.ActivationFunctionType.Exp
LNF = mybir.ActivationFunctionType.Ln
MULT = mybir.AluOpType.mult
ADD = mybir.AluOpType.add


def _runs(pairs):
    """Split [(e, j), ...] into maximal runs of consecutive e and j."""
    runs = []
    for e, j in pairs:
        if runs and runs[-1][0] + runs[-1][2] == e and runs[-1][1] + runs[-1][2] == j:
            runs[-1][2] += 1
        else:
            runs.append([e, j, 1])
    return runs


def _build_nc(routes_s: np.ndarray, gates_s: np.ndarray, esc: float):
    nc = bacc.Bacc("TRN2", target_bir_lowering=False, debug=False,
                   num_devices=NCORES)

    q_d = nc.dram_tensor("q", [NPROJ, PART, EC], F16, kind="ExternalInput")
    k_d = nc.dram_tensor("k", [NPROJ, PART, EC], F16, kind="ExternalInput")
    v_d = nc.dram_tensor("v", [NPROJ, PART, EC], F16, kind="ExternalInput")
    o_d = nc.dram_tensor("out", [PART, EC], BF16, kind="ExternalOutput")

    def runs_for(e_lo, e_hi):
        out = []
        for w in range(W):
            pairs = [(e, int(routes_s[e, w])) for e in range(e_lo, e_hi)]
            for e0, j0, L in _runs(pairs):
                out.append((w, e0, j0, L))
        return out

    with tile.TileContext(nc) as tc:
        with (
            tc.tile_pool(name="io", bufs=1) as io_p,
            tc.tile_pool(name="mid", bufs=1) as mid_p,
        ):
            qs = io_p.tile([PART, EC], F16, name="qs", tag="qs")
            ks = io_p.tile([PART, EC], F16, name="ks", tag="ks")
            vs = io_p.tile([PART, EC], F16, name="vs", tag="vs")
            kraw = io_p.tile([PART, EC], F16, name="kraw", tag="kraw")
            qraw = io_p.tile([PART, EC], F16, name="qraw", tag="qraw")
            vraw = io_p.tile([PART, EC], F16, name="vraw", tag="vraw")
            tp = mid_p.tile([PART, W * EC], F16, name="tp", tag="tp")
            epr = mid_p.tile([PART, 2 * W * EC], BF16, name="epr", tag="epr")
            dn = mid_p.tile([PART, 2 * EC], BF16, name="dn", tag="dn")
            lnt = mid_p.tile([PART, EC], F32, name="lnt", tag="lnt")
            rr = mid_p.tile([PART, EC], BF16, name="rr", tag="rr")
            og = mid_p.tile([PART, EC], BF16, name="og", tag="og")

            qv, kv, vv = q_d.ap(), k_d.ap(), v_d.ap()
            ov = o_d.ap()

            H = EC // 2
            crit_loads = []

            def load_cce(dst, src, half, ring):
                """proj0 plain + proj1 DMA-accumulate (needs warm Q7).
                Gated behind the critical first-wave loads so they don't
                pollute the DMA queues / completion semaphores."""
                c0, c1 = half * H, (half + 1) * H
                i0 = ring.dma_start(dst[:, c0:c1], src[0][:, c0:c1])
                for gi in crit_loads:
                    tile.add_dep_helper(i0.ins, gi.ins, sync=True,
                                        reason="load wave gating")
                nc.gpsimd.dma_start(dst[:, c0:c1], src[1][:, c0:c1],
                                    accum_op=ADD)

            def load_h1(raw, src, ring, crit=False, gated=False):
                """Single DMA for both projections of the h1 half into a
                raw tile - one completion semaphore, no Q7 on the path."""
                rv = raw[:].rearrange("p (n c) -> p n c", n=NPROJ)
                sv = src.rearrange("n p (h c) -> p h n c", h=2)[:, 1]
                i0 = ring.dma_start(rv, sv)
                if crit:
                    crit_loads.append(i0)
                if gated:
                    for gi in crit_loads:
                        tile.add_dep_helper(i0.ins, gi.ins, sync=True,
                                            reason="load wave gating")
                return i0

            def avg_h1(raw, dst):
                nc.vector.tensor_add(dst[:, H:EC], raw[:, 0:H],
                                     raw[:, H:2 * H])

            # strided views
            tpv = tp[:].rearrange("p (w c) -> p w c", w=W)
            epv = epr[:].rearrange("p (k w c) -> p k w c", k=2, w=W)
            dnv = dn[:].rearrange("p (k c) -> p k c", k=2)

            def score(e_lo, e_hi):
                """t = Qs * Ks[route], gate non-self slots, exp."""
                for w, e0, j0, L in runs_for(e_lo, e_hi):
                    nc.vector.tensor_mul(
                        tp[:, w * EC + e0 * COLS: w * EC + (e0 + L) * COLS],
                        qs[:, e0 * COLS:(e0 + L) * COLS],
                        ks[:, j0 * COLS:(j0 + L) * COLS])
                for w in range(1, W):
                    for e in range(e_lo, e_hi):
                        sl = slice(w * EC + e * COLS, w * EC + (e + 1) * COLS)
                        nc.vector.tensor_scalar_mul(
                            tp[:, sl], tp[:, sl], float(gates_s[e, w]))

            def expprod(e_lo, e_hi):
                """e = exp(esc * t) over all 3 slots, then prod = e * Vs."""
                c0, c1 = e_lo * COLS, e_hi * COLS
                nc.scalar.activation(epv[:, 0, :, c0:c1], tpv[:, :, c0:c1],
                                     EXPF, bias=0.0, scale=esc)
                for w, e0, j0, L in runs_for(e_lo, e_hi):
                    nc.vector.tensor_mul(
                        epr[:, (W + w) * EC + e0 * COLS:
                            (W + w) * EC + (e0 + L) * COLS],
                        epr[:, w * EC + e0 * COLS: w * EC + (e0 + L) * COLS],
                        vs[:, j0 * COLS:(j0 + L) * COLS])

            def finale(gc0, gc1, fin_chunks=1):
                fc = (gc1 - gc0) // fin_chunks
                for f in range(fin_chunks):
                    c0, c1 = gc0 + f * fc, gc0 + (f + 1) * fc
                    # den | num sums over w in two adds
                    nc.vector.tensor_add(dnv[:, :, c0:c1],
                                         epv[:, :, 0, c0:c1],
                                         epv[:, :, 1, c0:c1])
                    nc.vector.tensor_add(dnv[:, :, c0:c1], dnv[:, :, c0:c1],
                                         epv[:, :, 2, c0:c1])
                    # r = 0.5/den = exp(-ln(2*den)); 0.5 rides the ln scale
                    nc.scalar.activation(lnt[:, c0:c1], dn[:, c0:c1], LNF,
                                         bias=0.0, scale=2.0)
                    nc.scalar.activation(rr[:, c0:c1], lnt[:, c0:c1], EXPF,
                                         bias=0.0, scale=-1.0)
                    # out = num * r
                    nc.vector.tensor_mul(og[:, c0:c1],
                                         dn[:, EC + c0:EC + c1],
                                         rr[:, c0:c1])

            def store(c0, c1, ring=None):
                (ring or nc.sync).dma_start(ov[:, c0:c1], og[:, c0:c1])

            # Wave 1 (critical path, no Q7): one-DMA-per-tensor raw h1
            # loads + DVE averaging; unblocks groups 3 and 2.  The h0
            # halves ride CCE accumulate - by then the Q7 warmup (~6us)
            # has happened - gated behind wave 1.
            load_h1(kraw, kv, nc.sync, crit=True)
            load_h1(qraw, qv, nc.scalar, crit=True)
            # pin the exp+ln ACT table once the scalar ring's DMA is out
            nc.scalar.add_instruction(mybir.InstLoadActFuncSet(
                name=nc.get_next_instruction_name(),
                act_func_set_id=ACT_SET_LN_EXP, ins=[], outs=[]))
            load_h1(vraw, vv, nc.sync, gated=True)
            load_cce(ks, kv, 0, nc.sync)
            load_cce(qs, qv, 0, nc.scalar)
            load_cce(vs, vv, 0, nc.sync)
            avg_h1(kraw, ks)
            avg_h1(qraw, qs)
            # group 3 (experts 12-15): fully inside h1
            score(12, 16)
            avg_h1(vraw, vs)       # after score so DVE never stalls on V
            expprod(12, 16)
            finale(3 * GC, 4 * GC)
            # group 2
            score(8, 12)
            expprod(8, 12)
            finale(2 * GC, 3 * GC)
            store(2 * GC, 4 * GC)
            # group 0
            score(0, 4)
            expprod(0, 4)
            finale(0, GC)
            store(0, GC, nc.scalar)
            # group 1 - last: fine-grained to shorten the tail
            score(4, 8)
            expprod(4, 6)
            expprod(6, 8)
            finale(GC, 2 * GC, fin_chunks=2)
            store(GC, GC + GC // 2)
            store(GC + GC // 2, 2 * GC, nc.scalar)

    nc.compile()
    return nc


_cache: dict = {}


def _get_nc(routes_s: np.ndarray, gates_s: np.ndarray, esc: float):
    key = (routes_s.tobytes(), gates_s.tobytes(), float(esc))
    if key not in _cache:
        _cache[key] = _build_nc(routes_s, gates_s, esc)
    return _cache[key]


def _slot_sort(routes: np.ndarray, betas: np.ndarray):
    """Slot-permute so slot0 = self (gate 1); others sorted by offset."""
    gate = np.where(routes != np.arange(E, dtype=np.int32)[:, None],
                    1.0 / (1.0 + np.exp(-betas.astype(np.float64))),
                    1.0)
    routes_s = np.zeros((E, W), np.int32)
    gates_s = np.ones((E, W), np.float64)
    for e in range(E):
        slots = list(range(W))
        self_w = [w for w in slots if routes[e, w] == e]
        assert self_w, f"expert {e} missing self route"
        rest = [w for w in slots if w != self_w[0]]
        rest.sort(key=lambda w: int(routes[e, w]) - e)
        order = [self_w[0]] + rest
        routes_s[e] = routes[e, order]
        gates_s[e] = gate[e, order]
    return routes_s, gates_s.astype(np.float32)


def kernel(Q_proj, K_proj, V_proj, betas, temperature, routes, num_patches):
    Q = np.asarray(Q_proj, dtype=np.float32)
    K = np.asarray(K_proj, dtype=np.float32)
    V = np.asarray(V_proj, dtype=np.float32)
    betas = np.asarray(betas, dtype=np.float32)
    temp = np.asarray(temperature, dtype=np.float32)
    routes = np.asarray(routes, dtype=np.int32)
    assert int(num_patches) == E * P

    # Qs = Q0+Q1 (2x the mean); the 0.25 from both means is folded into
    # the exp scale esc together with sqrt(d)*|temperature|.
    esc = float(0.25 / (np.sqrt(np.float32(EXPERT_DIM)) * np.abs(temp[0])))
    routes_s, gates_s = _slot_sort(routes, betas)
    nc = _get_nc(routes_s, gates_s, esc)

    def prep(X):
        # [E, NPROJ, BS, P] -> [NPROJ, (b ph), (e c)] fp16
        return np.ascontiguousarray(
            X.reshape(E, NPROJ, BS, PH, COLS).transpose(1, 2, 3, 0, 4)
            .reshape(NPROJ, PART, EC).astype(np.float16))

    in_maps = []
    for c in range(NCORES):
        sl = slice(c * BS, (c + 1) * BS)
        in_maps.append({
            "q": prep(Q[:, :, sl, :]),
            "k": prep(K[:, :, sl, :]),
            "v": prep(V[:, :, sl, :]),
        })

    res = run_bass_kernel_spmd(nc, in_maps, list(range(NCORES)))
    out = np.empty((B, E * P), np.float32)
    for c in range(NCORES):
        o = np.asarray(res.results[c]["out"]).astype(np.float32)
        out[c * BS:(c + 1) * BS] = (
            o.reshape(BS, PH, E, COLS).transpose(0, 2, 1, 3)
            .reshape(BS, E * P))
    return out
